# revision 4
# baseline (speedup 1.0000x reference)
"""NT-Xent (SimCLR) loss on 8 Trainium2 NeuronCores — v3.

Full inputs z1, z2: [4096, 256] f32.  z = concat -> [8192, 256], rows
L2-normalized, sim = zn @ zn.T / 0.5 with the diagonal masked, row
log-softmax, loss = -mean over rows of logp[i, pair(i)].

v3 design (per core; data-parallel rows, every core sees all columns):
  - Host passes z1/z2 pre-cast to bf16 plus the core's own f32 row block
    (zrows) and positive-pair block (zpair).
  - The z^T moving operand comes straight from DRAM via
    dma_start_transpose (XBAR tiles, d = k*128 + p), killing all PE
    transposition of z.  PSUM belongs entirely to the matmul pipeline.
  - Column norms: squares+reduce on DVE (bf16) from a natural-layout
    copy, rsqrt via int bit-trick + 1 Newton step.  The per-column
    1/||z_j|| vector is flattened to one partition by a small SBUF->SBUF
    DMA (p-major order matches the transposed column order), replicated
    with gpsimd partition_broadcast, and applied by two bf16
    tensor_tensor multiplies per group that also cast to fp8.
  - Main matmul in fp8e4 with perf_mode=DoubleRow: [Ki=128, Ko=2, .]
    operands contract the full K=256 per instruction.
  - exp: sim_ii = 1/T = 2.0 is the row max, so denom_i =
    sum_j exp(2 cos_ij - 2) - exp(2 selfdot_i - 2); row norms fold into
    the activation scale.  Row-block SPLIT_PL uses the Schraudolph
    bit-trick exp on gpsimd + DVE reduce, the rest ScalarE Exp+accum.
  - log(denom) via the inverse bit-trick on DVE (no Ln table switch).
  - loss_row = 2 + log(denom_i - diag_i) - 2*pos_i; host averages rows.
"""

import numpy as np
from contextlib import ExitStack

import ml_dtypes

import concourse.bass as bass
import concourse.bacc as bacc
import concourse.mybir as mybir
import concourse.tile as tile
from concourse import masks
from concourse.bass import ts
from concourse.bass_utils import run_bass_kernel_spmd

F32 = mybir.dt.float32
I32 = mybir.dt.int32
BF16 = mybir.dt.bfloat16
FP8 = mybir.dt.float8e4
AF = mybir.ActivationFunctionType
ALU = mybir.AluOpType

P = 128          # partitions
D = 256          # feature dim
N = 4096         # rows per z1 / z2
R = 2 * N        # 8192 total rows
NCORES = 8
RPC = R // NCORES          # 1024 rows per core
NB = RPC // P              # 8 row blocks per core
NT = R // P                # 64 natural tiles of the full z
GROUPS = 4                 # process full z in 4 groups of 16 tiles
TPG = NT // GROUPS         # 16 tiles per group = 2048 sim columns
QT = 4                     # tiles per quarter (group-0 pipelined prologue)
SCALE = 2.0                # 1/temperature
MM_DT = BF16

SPLIT_PL = (7,)            # row-blocks whose exp runs on gpsimd+DVE
SPLIT_SC = tuple(b for b in range(NB) if b not in SPLIT_PL)

# Schraudolph: exp(y) ~ bitcast_f32(int32(EXP_A*y + EXP_B))
EXP_A = float(2 ** 23 / np.log(2.0))
EXP_B = float(127 * 2 ** 23 - 366393)
# inverse trick: ln(x) ~ (bitcast_i32(x) - LOG_B) / EXP_A, mean-centered
LOG_B = float(127 * 2 ** 23 - 0.0573 * 2 ** 23)


def _dve_rsqrt(nc, scratch, r_view, a_view, magic_view, n, tag, steps=2):
    """r = 1/sqrt(a) entirely on DVE: int bit-trick seed + Newton steps."""
    ri = r_view.bitcast(I32)
    ai = a_view.bitcast(I32)
    nc.vector.tensor_scalar(
        out=ri, in0=ai, scalar1=1, scalar2=None, op0=ALU.arith_shift_right
    )
    nc.vector.tensor_tensor(out=ri, in0=magic_view, in1=ri, op=ALU.subtract)
    for s in range(steps):
        t1 = scratch.tile([P, n], F32, tag=tag, bufs=2, name=f"{tag}_n{s}")
        nc.vector.tensor_tensor(out=t1[:], in0=r_view, in1=r_view, op=ALU.mult)
        nc.vector.tensor_tensor(out=t1[:], in0=t1[:], in1=a_view, op=ALU.mult)
        nc.vector.tensor_scalar(
            out=t1[:], in0=t1[:], scalar1=-0.5, scalar2=1.5,
            op0=ALU.mult, op1=ALU.add,
        )
        nc.vector.tensor_tensor(out=r_view, in0=r_view, in1=t1[:], op=ALU.mult)


def build_nc(loop_n=None, stage="full"):
    S = {"dma": 0, "stats": 1, "trans": 2, "mm": 3, "full": 4}[stage]
    nc = bacc.Bacc(None, target_bir_lowering=False, debug=False)

    z1b = nc.declare_dram_parameter("z1b", [N, D], BF16, isOutput=False)
    z2b = nc.declare_dram_parameter("z2b", [N, D], BF16, isOutput=False)
    z1c = nc.declare_dram_parameter("z1c", [N, D], BF16, isOutput=False)
    z2c = nc.declare_dram_parameter("z2c", [N, D], BF16, isOutput=False)
    zrows = nc.declare_dram_parameter("zrows", [RPC, D], F32, isOutput=False)
    zpair = nc.declare_dram_parameter("zpair", [RPC, D], F32, isOutput=False)
    out = nc.declare_dram_parameter("loss_rows", [NB, P], F32, isOutput=True)

    with tile.TileContext(nc) as tc, ExitStack() as ctx:
        consts = ctx.enter_context(tc.tile_pool(name="consts", bufs=1))
        small = ctx.enter_context(tc.tile_pool(name="small", bufs=1))
        scratch = ctx.enter_context(tc.tile_pool(name="scratch", bufs=2))
        zgp = ctx.enter_context(tc.tile_pool(name="zgp", bufs=2))
        ztp = ctx.enter_context(tc.tile_pool(name="ztp", bufs=2))
        znt_pool = ctx.enter_context(tc.tile_pool(name="znt", bufs=1))
        zr_pool = ctx.enter_context(tc.tile_pool(name="zrp", bufs=1))
        bcp = ctx.enter_context(tc.tile_pool(name="bcp", bufs=2))
        psum = ctx.enter_context(
            tc.tile_pool(name="psum", bufs=2, space=bass.MemorySpace.PSUM)
        )
        expout = ctx.enter_context(tc.tile_pool(name="expout", bufs=2))

        identity = consts.tile([P, P], F32)
        masks.make_identity(nc, identity[:])
        negtwo = consts.tile([P, 1], F32)
        nc.gpsimd.memset(negtwo[:], -2.0)
        magic = consts.tile([P, TPG], I32)
        nc.gpsimd.memset(magic[:], 0x5F3759DF)

        loop_cm = tc.For_i(0, loop_n, 1) if loop_n else ExitStack()
        ctx.enter_context(loop_cm)

        # Stage ALL loads up front, spread across the three DGE rings
        # (SP / Activation / Pool) — DMAs on one ring serialize on each
        # other's completion, so ring assignment is the prologue schedule.
        ztN = [ztp.tile([P, 2, TPG * P], BF16, tag=f"zt{g}", bufs=1,
                   name=f"zt{g}") for g in range(GROUPS)]
        zgN = [zgp.tile([P, TPG, D], BF16, tag=f"zg{g}", bufs=1,
                   name=f"zg{g}") for g in range(GROUPS)]
        zr = zr_pool.tile([P, NB, D], F32)
        zp = zr_pool.tile([P, NB, D], F32)

        def _src(g, natural=False):
            if natural:
                src = z1c if g < GROUPS // 2 else z2c
            else:
                src = z1b if g < GROUPS // 2 else z2b
            row0 = (g % (GROUPS // 2)) * (TPG * P)
            return src, row0

        # DMA lanes: SP/Act-issued DMAs round-robin over 8 DMAHW FIFO
        # lanes in EMISSION order; overlapping DRAM reads ALSO serialize,
        # so natural-layout slabs read the z1c/z2c twin copies.  Pool keeps
        # only the tiny flatten DMAs (own DMASW lanes).
        nc.sync.dma_start(zr[:], zrows.rearrange("(p r) d -> p r d", r=NB))
        for g in range(GROUPS):
            sn, r0_ = _src(g, natural=True)
            nc.sync.dma_start(
                zgN[g][:],
                sn[r0_ : r0_ + TPG * P, :].rearrange("(p r) d -> p r d",
                                                     r=TPG),
            )
            st, _ = _src(g)
            nc.sync.dma_start_transpose(ztN[g][:], st[r0_ : r0_ + TPG * P, :])
        nc.sync.dma_start(zp[:], zpair.rearrange("(p r) d -> p r d", r=NB))
        sa = small.tile([P, 2 * NB], F32)
        rnr2 = small.tile([P, NB], F32)      # 2/||row||  (exp scale)
        rnr2a = small.tile([P, NB], F32)     # EXP_A * rnr2 (schraudolph)
        rn2 = small.tile([P, 2 * NB], F32)
        rawpos = small.tile([P, NB], F32)

        sqf = small.tile([P, NT], BF16)
        sqf32 = small.tile([P, NT], F32)
        rnf = small.tile([P, NT], F32)
        rnb = small.tile([P, NT], BF16)

        if S >= 1:
            sqr = scratch.tile([P, NB, D], F32, tag="sqr", bufs=1, name="sqr")
            nc.gpsimd.tensor_tensor(out=sqr[:], in0=zr[:], in1=zr[:],
                                    op=ALU.mult)
            nc.vector.tensor_reduce(
                sa[:, 0:NB], sqr[:], axis=mybir.AxisListType.X, op=ALU.add
            )
            nc.vector.tensor_scalar_max(sa[:, 0:NB], sa[:, 0:NB], 1e-16)
            _dve_rsqrt(nc, scratch, rn2[:, 0:NB], sa[:, 0:NB], magic[:, 0:NB],
                       NB, "nwt_r")
            nc.vector.tensor_scalar_mul(rnr2[:], rn2[:, 0:NB], SCALE)
            nc.vector.tensor_scalar_mul(rnr2a[:], rnr2[:], EXP_A)

        # transpose raw row block -> zrawT [128, 2(k), 1024] in MM_DT
        zrawT = zr_pool.tile([P, 2, RPC], MM_DT)
        if S >= 2:
            ptr = psum.tile([P, 2, 2, 512], F32, tag="ps", name="ptr")
            for half in range(2):
                for j in range(4):
                    b = half * 4 + j
                    for k in range(2):
                        nc.tensor.transpose(
                            ptr[:, k, half, ts(j, P)], zr[:, b, ts(k, P)],
                            identity[:],
                        )
            nc.vector.tensor_copy(
                zrawT[:].rearrange("p k (h c) -> p k h c", c=512), ptr[:]
            )

        # ---- full z: 4 groups of 16 row-tiles -----------------------------
        znt = [
            znt_pool.tile([P, 2, TPG * P], MM_DT, tag=f"znt{g}", name=f"znt{g}")
            for g in range(GROUPS)
        ]
        denoms = small.tile([P, NB, GROUPS], F32)

        def col_stats(zg, t_lo, ti_lo, nst, tag):
            """norms for tiles [t_lo, t_lo+nst): squares+reduce+rsqrt (DVE)."""
            ssl = slice(t_lo, t_lo + nst)
            tis = slice(ti_lo, ti_lo + nst)
            sqg = scratch.tile([P, nst, D], BF16, tag="sqg", bufs=2,
                               name=f"sqg{tag}")
            nc.vector.tensor_tensor(out=sqg[:], in0=zg[:, tis, :],
                                    in1=zg[:, tis, :], op=ALU.mult)
            with nc.allow_low_precision(reason="bf16 colnorm^2 ok"):
                nc.vector.tensor_reduce(
                    sqf[:, ssl], sqg[:], axis=mybir.AxisListType.X, op=ALU.add
                )
            nc.vector.tensor_scalar_max(sqf32[:, ssl], sqf[:, ssl], 1e-16)
            _dve_rsqrt(nc, scratch, rnf[:, ssl], sqf32[:, ssl],
                       magic[:, 0:nst], nst, "nwt_g", steps=1)
            nc.vector.tensor_copy(rnb[:, ssl], rnf[:, ssl])

        def col_normalize(ztsrc, zdst, t_lo, nst, tag, src_c0=None):
            """flatten rn (p-major) -> broadcast -> scale+cast columns."""
            ssl = slice(t_lo, t_lo + nst)
            ncols = nst * P
            c0 = (t_lo % TPG) * P
            s0 = c0 if src_c0 is None else src_c0
            rnflat = bcp.tile([1, P, nst], BF16, tag=f"rnf1{tag}", bufs=1,
                              name=f"rnf1{tag}")
            nc.gpsimd.dma_start(rnflat[:], rnb[:, ssl])
            rnbc = bcp.tile([P, nst * P], BF16, tag=f"rnbc{tag}", bufs=1,
                            name=f"rnbc{tag}")
            nc.gpsimd.partition_broadcast(
                rnbc[:], rnflat[:].rearrange("a p t -> a (p t)")
            )
            for k in range(2):
                nc.vector.tensor_tensor(
                    out=zdst[:, k, c0 : c0 + ncols],
                    in0=ztsrc[:, k, s0 : s0 + ncols],
                    in1=rnbc[:], op=ALU.mult,
                )

        # ---- stats + normalize ladders for ALL groups (dataflow-timed) ----
        for grp in range(GROUPS):
            t0 = grp * TPG
            nq = 1
            nst = TPG // nq
            for sq_ in range(nq):
                if S >= 1:
                    col_stats(zgN[grp], t0 + sq_ * nst, sq_ * nst, nst,
                              f"g{grp}_{sq_}")
                if S >= 2:
                    col_normalize(ztN[grp], znt[grp], t0 + sq_ * nst, nst,
                                  f"g{grp}_{sq_}")

        # ---- matmul + exp: 4 groups x 8 row-blocks ------------------------
        for grp in range(GROUPS):
            for b in range(NB):
                if S >= 3:
                    pm = psum.tile([P, 4, 512], F32, tag="ps",
                                   name=f"pm{grp}_{b}")
                    if MM_DT == FP8:
                        for q in range(4):
                            nc.tensor.matmul(
                                pm[:, q, :],
                                zrawT[:, :, ts(b, P)],
                                znt[grp][:, :, ts(q, 512)],
                                start=True,
                                stop=True,
                                perf_mode=mybir.MatmulPerfMode.DoubleRow,
                            )
                    else:
                        for k in range(2):
                            for q in range(4):
                                nc.tensor.matmul(
                                    pm[:, q, :],
                                    zrawT[:, k, ts(b, P)],
                                    znt[grp][:, k, ts(q, 512)],
                                    start=(k == 0),
                                    stop=(k == 1),
                                )
                if S >= 4:
                    if b in SPLIT_SC:
                        eo = expout.tile([P, 4, 512], BF16, tag="eo", bufs=2,
                                         name=f"eo{grp}_{b}")
                        nc.scalar.activation(
                            eo[:], pm[:], AF.Exp,
                            bias=negtwo[:], scale=rnr2[:, b : b + 1],
                            accum_out=denoms[:, b, grp : grp + 1],
                        )
                    else:
                        eo2 = expout.tile([P, 4, 512], I32, tag="eo2",
                                          bufs=2, name=f"eo2{grp}_{b}")
                        nc.gpsimd.tensor_scalar(
                            out=eo2[:], in0=pm[:],
                            scalar1=rnr2a[:, b : b + 1],
                            scalar2=EXP_B - 2.0 * EXP_A,
                            op0=ALU.mult, op1=ALU.add,
                        )
                        nc.vector.tensor_reduce(
                            denoms[:, b, grp : grp + 1],
                            eo2[:].bitcast(F32).rearrange("p q c -> p (q c)"),
                            axis=mybir.AxisListType.X, op=ALU.add,
                        )

        # pair stats (epilogue inputs) — Pool/DVE, off the critical path
        if S >= 1:
            sqp = scratch.tile([P, NB, D], F32, tag="sqr", bufs=1,
                               name="sqzp")
            nc.vector.tensor_tensor(out=sqp[:], in0=zp[:], in1=zp[:],
                                    op=ALU.mult)
            nc.vector.tensor_reduce(
                sa[:, NB:], sqp[:], axis=mybir.AxisListType.X, op=ALU.add
            )
            posm = scratch.tile([P, NB, D], F32, tag="posm", bufs=1,
                                name="posm")
            nc.vector.tensor_tensor(out=posm[:], in0=zr[:], in1=zp[:],
                                    op=ALU.mult)
            nc.vector.tensor_reduce(
                rawpos[:], posm[:], axis=mybir.AxisListType.X, op=ALU.add
            )
            nc.vector.tensor_scalar_max(sa[:, NB:], sa[:, NB:], 1e-16)
            _dve_rsqrt(nc, scratch, rn2[:, NB:], sa[:, NB:],
                       magic[:, 0:NB], NB, "nwt_p")
            sd = small.tile([P, NB], F32)
            nc.vector.tensor_tensor(out=sd[:], in0=sa[:, 0:NB],
                                    in1=rn2[:, 0:NB], op=ALU.mult)
            nc.vector.tensor_tensor(out=sd[:], in0=sd[:], in1=rn2[:, 0:NB],
                                    op=ALU.mult)
            diag = small.tile([P, NB], F32)
            nc.scalar.activation(diag[:], sd[:], AF.Exp, bias=negtwo[:],
                                 scale=SCALE)
            posx = small.tile([P, NB], F32)
            nc.vector.tensor_tensor(out=posx[:], in0=rawpos[:],
                                    in1=rn2[:, 0:NB], op=ALU.mult)
            nc.vector.tensor_tensor(out=posx[:], in0=posx[:],
                                    in1=rn2[:, NB:], op=ALU.mult)

        # ---- epilogue: per-row loss ---------------------------------------
        if S < 4:
            outsb0 = small.tile([NB, P], F32)
            nc.gpsimd.memset(outsb0[:], 0.0)
            nc.sync.dma_start(out[:, :], outsb0[:])
        else:
            denom = small.tile([P, NB], F32)
            nc.vector.tensor_reduce(
                denom[:], denoms[:], axis=mybir.AxisListType.X, op=ALU.add
            )
            nc.vector.tensor_tensor(out=denom[:], in0=denom[:], in1=diag[:],
                                    op=ALU.subtract)
            # ln(denom) via inverse bit-trick (no act-table switch)
            logd = small.tile([P, NB], F32)
            nc.vector.tensor_scalar(
                out=logd[:], in0=denom[:].bitcast(I32),
                scalar1=1.0 / EXP_A, scalar2=-LOG_B / EXP_A,
                op0=ALU.mult, op1=ALU.add,
            )
            loss = small.tile([P, NB], F32)
            nc.vector.tensor_scalar_mul(loss[:], posx[:], -2.0)
            nc.vector.tensor_tensor(out=loss[:], in0=loss[:], in1=logd[:],
                                    op=ALU.add)
            nc.vector.tensor_scalar_add(loss[:], loss[:], 2.0)

            pl = psum.tile([P, 4, 512], F32, tag="ps")
            nc.tensor.transpose(pl[0:NB, 0, 0:P], loss[:], identity[:])
            outsb = small.tile([NB, P], F32)
            nc.vector.tensor_copy(outsb[:], pl[0:NB, 0, 0:P])
            nc.sync.dma_start(out[:, :], outsb[:])

    nc.compile()
    return nc


_NC = None


def _get_nc():
    global _NC
    if _NC is None:
        _NC = build_nc()
    return _NC


def _in_maps(z1, z2):
    z1 = np.ascontiguousarray(z1, dtype=np.float32)
    z2 = np.ascontiguousarray(z2, dtype=np.float32)
    z1b = z1.astype(ml_dtypes.bfloat16)
    z2b = z2.astype(ml_dtypes.bfloat16)
    z = np.concatenate([z1, z2], axis=0)
    maps = []
    for c in range(NCORES):
        lo = c * RPC
        plo = (lo + N) % R
        maps.append(
            {
                "z1b": z1b,
                "z2b": z2b,
                "z1c": z1b.copy(),
                "z2c": z2b.copy(),
                "zrows": np.ascontiguousarray(z[lo : lo + RPC]),
                "zpair": np.ascontiguousarray(z[plo : plo + RPC]),
            }
        )
    return maps


def run(z1, z2, trace=False, **kwargs):
    nc = _get_nc()
    res = run_bass_kernel_spmd(
        nc, _in_maps(z1, z2), list(range(NCORES)), trace=trace, **kwargs
    )
    rows = np.concatenate(
        [np.asarray(res.results[c]["loss_rows"]).reshape(-1) for c in range(NCORES)]
    )
    return np.float32(rows.mean()), res


def kernel(z1, z2):
    loss, _ = run(z1, z2)
    return loss
